# revision 1
# baseline (speedup 1.0000x reference)
"""Depthwise causal Conv1D (B=4, C=4096, L=4096, K=4) on 8 trn2 NeuronCores.

Sharding: channel-parallel (tensor parallel) — core i owns channels
[i*512, (i+1)*512). Depthwise conv has zero cross-channel interaction, so
there is no communication; each core computes its channel slab end to end.

Per-core kernel layout: channels on SBUF partitions (128 at a time), time on
the free dim. The 4-tap causal FIR along the free dim is computed as four
shifted multiply-accumulate passes with per-partition (per-channel) scalar
weights, split across three engines so no single engine is the bottleneck:

  ScalarE : out[3:L+3]  = w0 * x + bias   (activation, per-partition scale+bias)
            out[0:3]    = bias
  GPSIMD  : out[0:L]   += w3 * x          (scalar_tensor_tensor)
  VectorE : out[1:L+1] += w2 * x          (scalar_tensor_tensor)
            out[2:L+2] += w1 * x          (scalar_tensor_tensor)

DMA (HWDGE via nc.sync) streams 128x4096 fp32 tiles in and 128x4099 tiles
out; the kernel is HBM-bandwidth bound (~64 MB per core total traffic).
"""

import numpy as np

import concourse.bass as bass
import concourse.tile as tile
from concourse import bacc, mybir
from concourse.bass_utils import run_bass_kernel_spmd

B, C, L, K = 4, 4096, 4096, 4
PAD = K - 1
LOUT = L + PAD  # 4099
NCORES = 8
CS = C // NCORES  # 512 channels per core
DT = mybir.dt.float32

_AF = mybir.ActivationFunctionType
_OP = mybir.AluOpType


def build_nc(b=B, cs=CS, l=L, k=K, n_bufs=5, n_edge_chunks=4, pe_cols=2048):
    """Build the per-core Bass program. Parameterized for small-size sim tests.

    Channels on partitions, time on the free dim. x is loaded into a
    [128, pad + l + pad] tile with `pad` zero columns at both ends
    (xp[i] = x[i - pad]), so every tap reads in-bounds and the causal
    zero-padding falls out of the zero stuffing.

    Work split per tile:
      ScalarE : out[pad:lout] = w0 * xp[pad:lout] + bias; head cols = bias
      PE      : taps 1..k-1 for out cols [0, pe_cols) via diagonal weight
                matmuls accumulating in PSUM (out[m] += sum_t w_t*xp[m+t]),
                512-col chunks, fp32
      VectorE : PSUM chunks merged into out (tensor_tensor add), and
                taps 1..k-1 for out cols [pe_cols, lout) via fused
                scalar_tensor_tensor ops
    Stores issue from ScalarE's HWDGE, deferred one iteration; loads from
    SP. GpSimd stays idle (its tensor ops serialize against VectorE on the
    shared SBUF port pair).

    The first and last tiles are split column-wise into `n_edge_chunks`
    pieces (DVE-only taps) to shorten the pipeline ramp and drain.
    """
    ng = cs // 128
    pad = k - 1
    lout = l + pad
    wx = l + 2 * pad  # padded x width
    assert pe_cols % 512 == 0 and pe_cols + pad <= l

    nc = bacc.Bacc("TRN2", target_bir_lowering=False, debug=False, num_devices=NCORES)
    x_d = nc.dram_tensor("x", [b, cs, l], DT, kind="ExternalInput").ap()
    # packed per-channel constants: wb[c] = [w_0..w_{k-1}, bias]
    wb_d = nc.dram_tensor("wb", [cs, k + 1], DT, kind="ExternalInput").ap()
    eye_d = nc.dram_tensor("eye", [128, 128], DT, kind="ExternalInput").ap()
    o_d = nc.dram_tensor("out", [b, cs, lout], DT, kind="ExternalOutput").ap()

    with tile.TileContext(nc) as tc:
        with (
            tc.tile_pool(name="consts", bufs=1) as cpool,
            tc.tile_pool(name="xs", bufs=n_bufs) as xpool,
            tc.tile_pool(name="os", bufs=n_bufs) as opool,
            tc.tile_pool(name="ps", bufs=8, space="PSUM") as ppool,
        ):
            # Constants are emitted lazily (after the first x chunk load) so
            # the first compute tile's data leads the SP DMA trigger queue.
            consts = []
            diags = {}

            def emit_consts():
                # Per-group constant columns: [128, k+1] = w_0..w_{k-1}, bias.
                for g in range(ng):
                    ct = cpool.tile([128, k + 1], DT, tag=f"c{g}")
                    nc.sync.dma_start(ct[:], wb_d[g * 128 : (g + 1) * 128, :])
                    consts.append(ct)
                # identity and per-(group, tap) diagonal weight matrices for PE
                if pe_cols > 0:
                    ident = cpool.tile([128, 128], DT, tag="eye")
                    nc.sync.dma_start(ident[:], eye_d[:])
                    for g in range(ng):
                        for t in range(1, k):
                            dg = cpool.tile([128, 128], DT, tag=f"d{g}_{t}")
                            nc.vector.tensor_scalar(
                                out=dg[:], in0=ident[:],
                                scalar1=consts[g][:, t : t + 1],
                                scalar2=None, op0=_OP.mult,
                            )
                            diags[(g, t)] = dg

            n_tiles = b * ng
            pending_stores = []  # deferred to keep ACT's HWDGE queue unblocked

            def flush_stores():
                for dst, src in pending_stores:
                    nc.scalar.dma_start(dst, src)
                pending_stores.clear()

            ti = 0
            for bi in range(b):
                for g in range(ng):
                    c0 = g * 128
                    first, last = ti == 0, ti == n_tiles - 1
                    edge = first or last
                    nchunk = n_edge_chunks if edge else 1
                    cw = l // nchunk
                    n_pe = 0 if edge else pe_cols  # edge tiles are DVE-only

                    xt = xpool.tile([128, wx], DT, tag="x")
                    # zero stuffing: xp[0:pad] = xp[pad+l:] = 0 (GpSimd: tiny,
                    # keeps the VectorE queue free of slot-recycle waits)
                    nc.gpsimd.memset(xt[:, 0:pad], 0.0)
                    nc.gpsimd.memset(xt[:, pad + l : wx], 0.0)
                    if first:
                        # chunk 0 load leads the SP queue; consts follow it
                        nc.sync.dma_start(
                            xt[:, pad : pad + cw], x_d[bi, c0 : c0 + 128, 0:cw]
                        )
                        emit_consts()
                        for c in range(1, nchunk):
                            nc.sync.dma_start(
                                xt[:, pad + c * cw : pad + (c + 1) * cw],
                                x_d[bi, c0 : c0 + 128, c * cw : (c + 1) * cw],
                            )
                    else:
                        nc.sync.dma_start(
                            xt[:, pad : pad + l], x_d[bi, c0 : c0 + 128, :]
                        )
                    ot = opool.tile([128, lout], DT, tag="o")
                    ct = consts[g]

                    for c in range(nchunk):
                        j0, j1 = c * cw, (c + 1) * cw
                        # tap 0 (+bias): out[pad+j] = w0*x[j] + bias  (ScalarE)
                        nc.scalar.activation(
                            ot[:, pad + j0 : pad + j1],
                            xt[:, pad + j0 : pad + j1], _AF.Identity,
                            bias=ct[:, k : k + 1], scale=ct[:, 0:1],
                        )
                        if c == 0:
                            # head columns [0:pad] = bias  (ScalarE)
                            nc.scalar.activation(
                                ot[:, 0:pad], xt[:, 0:pad], _AF.Identity,
                                bias=ct[:, k : k + 1], scale=0.0,
                            )
                            flush_stores()
                        # PE portion: out[m] += sum_t w_t * xp[m+t], m in [0, n_pe)
                        if c == 0 and n_pe > 0:
                            for m0 in range(0, n_pe, 512):
                                pt = ppool.tile([128, 512], DT, tag="p")
                                for t in range(1, k):
                                    nc.tensor.matmul(
                                        pt[:], lhsT=diags[(g, t)][:],
                                        rhs=xt[:, m0 + t : m0 + t + 512],
                                        start=(t == 1), stop=(t == k - 1),
                                    )
                                nc.vector.tensor_tensor(
                                    out=ot[:, m0 : m0 + 512],
                                    in0=pt[:], in1=ot[:, m0 : m0 + 512], op=_OP.add,
                                )
                        # DVE taps: out[m] += w_t * xp[m+t].
                        # On edge tiles, chunk c handles out [j0-pad, j1-pad)
                        # so its tap reads stay within x chunks <= c (xp idx
                        # m+t <= j1-1), keeping the ramp free of forward deps.
                        if edge:
                            m_lo = 0 if c == 0 else j0 - pad
                            m_hi = lout if c == nchunk - 1 else j1 - pad
                        else:
                            m_lo = max(j0, n_pe)
                            m_hi = lout if c == nchunk - 1 else j1
                        if m_hi > m_lo:
                            for t in range(k - 1, 0, -1):
                                nc.vector.scalar_tensor_tensor(
                                    out=ot[:, m_lo:m_hi],
                                    in0=xt[:, m_lo + t : m_hi + t],
                                    scalar=ct[:, t : t + 1],
                                    in1=ot[:, m_lo:m_hi],
                                    op0=_OP.mult, op1=_OP.add,
                                )
                        if last:
                            # store exactly the finalized range of this chunk
                            nc.scalar.dma_start(
                                o_d[bi, c0 : c0 + 128, m_lo:m_hi], ot[:, m_lo:m_hi]
                            )
                    if not last:
                        pending_stores.append((o_d[bi, c0 : c0 + 128, :], ot[:]))
                    ti += 1
            flush_stores()
    nc.compile()
    return nc


_cached_nc = None


def _get_nc():
    global _cached_nc
    if _cached_nc is None:
        _cached_nc = build_nc()
    return _cached_nc


def run(x, kernel, bias, trace=False, **kwargs):
    """Shard, run on 8 cores, gather. Returns (out, BassKernelResults)."""
    x = np.ascontiguousarray(x, dtype=np.float32)
    w = np.asarray(kernel, dtype=np.float32).reshape(K, C)
    bvec = np.asarray(bias, dtype=np.float32).reshape(C)
    # wb[c] = [w_0[c] .. w_{K-1}[c], bias[c]]
    wb = np.concatenate([w.T, bvec[:, None]], axis=1).astype(np.float32)

    eye = np.eye(128, dtype=np.float32)
    in_maps = []
    for i in range(NCORES):
        sl = slice(i * CS, (i + 1) * CS)
        in_maps.append(
            {
                "x": np.ascontiguousarray(x[:, sl, :]),
                "wb": np.ascontiguousarray(wb[sl, :]),
                "eye": eye,
            }
        )

    nc = _get_nc()
    bkr = run_bass_kernel_spmd(
        nc, in_maps, core_ids=list(range(NCORES)), trace=trace, **kwargs
    )
    out = np.concatenate([r["out"] for r in bkr.results], axis=1)
    return out, bkr


def kernel(x, kernel, bias):
    import os

    prev = os.environ.get("BASS_NEVER_TRACE")
    os.environ["BASS_NEVER_TRACE"] = "1"  # keep the runner off the NTFF path
    try:
        out, _ = run(x, kernel, bias)
    finally:
        if prev is None:
            os.environ.pop("BASS_NEVER_TRACE", None)
        else:
            os.environ["BASS_NEVER_TRACE"] = prev
    return out



# revision 2
# speedup vs baseline: 1.6014x; 1.6014x over previous
"""Depthwise causal Conv1D (B=4, C=4096, L=4096, K=4) on 8 trn2 NeuronCores.

Sharding: channel-parallel (tensor parallel) — core i owns channels
[i*512, (i+1)*512). Depthwise conv has zero cross-channel interaction, so
there is no communication; each core computes its channel slab end to end.

The kernel is HBM-bandwidth bound, so all HBM I/O is bf16: x is converted
to bf16 on the host, streamed in at half the fp32 byte count, and the
output is stored bf16 and widened to fp32 on the host (the 2e-2 rel-err
budget dwarfs bf16's ~2^-9 rounding).

Per-core layout: channels on SBUF partitions (128 at a time), time on the
free dim. x lives in a [128, 3+L+3] tile with zero pads so out[m] =
sum_t w_t * xp[m+t]. The 4-tap FIR is split per 512-column chunk (PSUM
bank width) across three engines:

  PE      : taps 0,1,3 — diagonal-weight bf16 matmuls accumulating in
            PSUM (the odd-offset taps must avoid DVE: its 2x bf16
            packing needs 4B-aligned reads)
  ScalarE : out_bf16 = psum + bias   (activation, per-partition bias,
            reads PSUM, converts to bf16)
  VectorE : out_bf16 += w2 * xp[m+2] (scalar_tensor_tensor; +2 elements
            keeps the bf16 read 4B-aligned so the 2x mode engages)
  GpSimd  : zero-stuffs the x pads

Loads issue from sync's HWDGE, stores from ScalarE's (deferred one tile);
diag weight matrices are precomputed on the host.
"""

import numpy as np

import concourse.bass as bass
import concourse.tile as tile
from concourse import bacc, mybir
from concourse.bass_utils import run_bass_kernel_spmd

B, C, L, K = 4, 4096, 4096, 4
PAD = K - 1
LOUT = L + PAD  # 4099
NCORES = 8
CS = C // NCORES  # 512 channels per core
NG = CS // 128  # 4 partition groups per core
F32 = mybir.dt.float32
BF16 = mybir.dt.bfloat16

PE_TAPS = (0, 1, 3)  # taps accumulated on PE via diag matmuls
DVE_TAP = 2  # tap fused into the final DVE pass (4B-aligned in bf16)

_AF = mybir.ActivationFunctionType
_OP = mybir.AluOpType


def _chunks(l=L, lout=LOUT):
    """512-col chunks covering [0, lout): 7x512 then the 515 tail split
    258+257 so every chunk fits one PSUM bank and starts 4B-aligned."""
    out = []
    m0 = 0
    while lout - m0 > 515:
        out.append((m0, 512))
        m0 += 512
    rest = lout - m0
    a = (rest + 1) // 2
    a += a % 2  # keep the second chunk's start even (4B-aligned bf16)
    out.append((m0, a))
    out.append((m0 + a, rest - a))
    return out


def build_nc(b=B, cs=CS, l=L, k=K, n_bufs=5, n_load_chunks=4):
    ng = cs // 128
    pad = k - 1
    lout = l + pad
    wx = l + 2 * pad  # padded x width

    nc = bacc.Bacc("TRN2", target_bir_lowering=False, debug=False, num_devices=NCORES)
    x_d = nc.dram_tensor("x", [b, cs, l], BF16, kind="ExternalInput").ap()
    # per-(group, tap) diagonal weight matrices, taps PE_TAPS order
    dg_d = nc.dram_tensor("dg", [ng * len(PE_TAPS), 128, 128], BF16,
                          kind="ExternalInput").ap()
    # packed per-channel constants: ct[c] = [w_DVE_TAP, bias]
    ct_d = nc.dram_tensor("ct", [cs, 2], F32, kind="ExternalInput").ap()
    o_d = nc.dram_tensor("out", [b, cs, lout], BF16, kind="ExternalOutput").ap()

    chunks = _chunks(l, lout)

    with tile.TileContext(nc) as tc:
        with (
            tc.tile_pool(name="consts", bufs=1) as cpool,
            tc.tile_pool(name="xs", bufs=n_bufs) as xpool,
            tc.tile_pool(name="os", bufs=n_bufs) as opool,
            tc.tile_pool(name="ps", bufs=8, space="PSUM") as ppool,
        ):
            consts = []
            diags = {}

            def emit_consts():
                for g in range(ng):
                    ct = cpool.tile([128, 2], F32, tag=f"c{g}")
                    nc.sync.dma_start(ct[:], ct_d[g * 128 : (g + 1) * 128, :])
                    consts.append(ct)
                for g in range(ng):
                    for j, t in enumerate(PE_TAPS):
                        dgt = cpool.tile([128, 128], BF16, tag=f"d{g}_{t}")
                        nc.sync.dma_start(
                            dgt[:], dg_d[g * len(PE_TAPS) + j]
                        )
                        diags[(g, t)] = dgt

            n_tiles = b * ng
            pending_stores = []  # deferred to keep ACT's HWDGE queue unblocked

            def flush_stores():
                for dst, src in pending_stores:
                    nc.scalar.dma_start(dst, src)
                pending_stores.clear()

            ti = 0
            for bi in range(b):
                for g in range(ng):
                    c0 = g * 128
                    first, last = ti == 0, ti == n_tiles - 1

                    xt = xpool.tile([128, wx], BF16, tag="x")
                    nc.gpsimd.memset(xt[:, 0:pad], 0.0)
                    nc.gpsimd.memset(xt[:, pad + l : wx], 0.0)
                    if first:
                        # chunked load so compute ramps before the full
                        # tile lands; consts follow the first chunk
                        cw = l // n_load_chunks
                        nc.sync.dma_start(
                            xt[:, pad : pad + cw], x_d[bi, c0 : c0 + 128, 0:cw]
                        )
                        emit_consts()
                        for c in range(1, n_load_chunks):
                            nc.sync.dma_start(
                                xt[:, pad + c * cw : pad + (c + 1) * cw],
                                x_d[bi, c0 : c0 + 128, c * cw : (c + 1) * cw],
                            )
                    else:
                        nc.sync.dma_start(
                            xt[:, pad : pad + l], x_d[bi, c0 : c0 + 128, :]
                        )
                    ot = opool.tile([128, lout], BF16, tag="o")
                    ct = consts[g]

                    for m0, fd in chunks:
                        pt = ppool.tile([128, 512], F32, tag="p")
                        for j, t in enumerate(PE_TAPS):
                            nc.tensor.matmul(
                                pt[:, 0:fd], lhsT=diags[(g, t)][:],
                                rhs=xt[:, m0 + t : m0 + t + fd],
                                start=(j == 0), stop=(j == len(PE_TAPS) - 1),
                            )
                        # out = psum + bias (converts to bf16)
                        nc.scalar.activation(
                            ot[:, m0 : m0 + fd], pt[:, 0:fd], _AF.Identity,
                            bias=ct[:, 1:2], scale=1.0,
                        )
                        if m0 == 0:
                            flush_stores()
                        # out += w2 * xp[m+2]
                        nc.vector.scalar_tensor_tensor(
                            out=ot[:, m0 : m0 + fd],
                            in0=xt[:, m0 + DVE_TAP : m0 + DVE_TAP + fd],
                            scalar=ct[:, 0:1],
                            in1=ot[:, m0 : m0 + fd],
                            op0=_OP.mult, op1=_OP.add,
                        )
                        if last:
                            nc.scalar.dma_start(
                                o_d[bi, c0 : c0 + 128, m0 : m0 + fd],
                                ot[:, m0 : m0 + fd],
                            )
                    if not last:
                        pending_stores.append((o_d[bi, c0 : c0 + 128, :], ot[:]))
                    ti += 1
            flush_stores()
    nc.compile()
    return nc


_cached_nc = None


def _get_nc():
    global _cached_nc
    if _cached_nc is None:
        _cached_nc = build_nc()
    return _cached_nc


def run(x, kernel, bias, trace=False, **kwargs):
    """Shard, run on 8 cores, gather. Returns (out, BassKernelResults)."""
    import ml_dtypes

    bf16 = ml_dtypes.bfloat16
    x_bf = np.ascontiguousarray(np.asarray(x, dtype=np.float32)).astype(bf16)
    w = np.asarray(kernel, dtype=np.float32).reshape(K, C)
    bvec = np.asarray(bias, dtype=np.float32).reshape(C)
    # ct[c] = [w_DVE_TAP[c], bias[c]]
    ct = np.stack([w[DVE_TAP], bvec], axis=1).astype(np.float32)

    w_bf = w.astype(bf16)
    in_maps = []
    for i in range(NCORES):
        sl = slice(i * CS, (i + 1) * CS)
        dg = np.zeros((NG * len(PE_TAPS), 128, 128), dtype=bf16)
        for g in range(NG):
            for j, t in enumerate(PE_TAPS):
                np.fill_diagonal(
                    dg[g * len(PE_TAPS) + j],
                    w_bf[t, i * CS + g * 128 : i * CS + (g + 1) * 128],
                )
        in_maps.append(
            {
                "x": np.ascontiguousarray(x_bf[:, sl, :]),
                "dg": dg,
                "ct": np.ascontiguousarray(ct[sl, :]),
            }
        )

    nc = _get_nc()
    bkr = run_bass_kernel_spmd(
        nc, in_maps, core_ids=list(range(NCORES)), trace=trace, **kwargs
    )
    out = np.concatenate(
        [r["out"].astype(np.float32) for r in bkr.results], axis=1
    )
    return out, bkr


def kernel(x, kernel, bias):
    import os

    prev = os.environ.get("BASS_NEVER_TRACE")
    os.environ["BASS_NEVER_TRACE"] = "1"  # keep the runner off the NTFF path
    try:
        out, _ = run(x, kernel, bias)
    finally:
        if prev is None:
            os.environ.pop("BASS_NEVER_TRACE", None)
        else:
            os.environ["BASS_NEVER_TRACE"] = prev
    return out
